# revision 35
# baseline (speedup 1.0000x reference)
"""Raw-bass (no TileContext) variant of the DiagonalUpsample kernel.

Same algorithm as kernel.py (bf16, fused u32 shift-or interleave, Act/DVE
cast split, phase-ordered DMA) but with hand-wired semaphores instead of
the tile framework, saving the tc entry/exit cascades (~2 us measured on
minimal kernels).
"""

import numpy as np

import concourse.bass as bass
from concourse import bacc, mybir
from concourse.bass_utils import run_bass_kernel_spmd

B, C, H, W = 16, 3, 512, 512
N_CORES = 8
B_LOC = B // N_CORES
ROWS = B_LOC * C * H
P = 128
K = 6
N_TILES = ROWS // (P * K)
U16 = mybir.dt.uint16
U32 = mybir.dt.uint32

_nc_cache = []
TRACE = False
LAST_RESULT = None


def _stt_u32(nc, out, in0, shift, in1):
    eng = nc.vector
    return eng.add_instruction(
        mybir.InstTensorScalarPtr(
            name=nc.get_next_instruction_name(),
            is_scalar_tensor_tensor=True,
            op0=mybir.AluOpType.logical_shift_left,
            op1=mybir.AluOpType.bitwise_or,
            ins=[
                eng.lower_ap(in0),
                mybir.ImmediateValue(dtype=mybir.dt.uint32, value=shift),
                eng.lower_ap(in1),
            ],
            outs=[eng.lower_ap(out)],
        )
    )


def _build_nc() -> bass.Bass:
    nc = bacc.Bacc("TRN2", debug=False)
    up = nc.dram_tensor("up", [2, P, K * W], U16, kind="ExternalInput")
    down = nc.dram_tensor("down", [2, P, K * W], U16, kind="ExternalInput")
    upc = nc.dram_tensor("upc", [P, 2 * K * W], U16, kind="ExternalInput")
    downc = nc.dram_tensor("downc", [P, 2 * K * W], U16, kind="ExternalInput")
    out = nc.dram_tensor("out", [N_TILES, P, K * 2 * W], U32,
                         kind="ExternalOutput")

    # static SBUF layout (per partition): 48KB inputs + 3x24KB cast sets
    # + 3x24KB out sets = 192KB
    u_in = [nc.alloc_sbuf_tensor(f"u{t}", [P, K * W], U16) for t in range(2)]
    d_in = [nc.alloc_sbuf_tensor(f"d{t}", [P, K * W], U16) for t in range(2)]
    uc_in = nc.alloc_sbuf_tensor("uc", [P, 2 * K * W], U16)
    dc_in = nc.alloc_sbuf_tensor("dc", [P, 2 * K * W], U16)
    cast_u = [nc.alloc_sbuf_tensor(f"cu{s}", [P, K * W], U32) for s in range(3)]
    cast_d = [nc.alloc_sbuf_tensor(f"cd{s}", [P, K * W], U32) for s in range(3)]
    o_buf = [nc.alloc_sbuf_tensor(f"o{s}", [P, K * 2 * W], U32) for s in range(3)]

    load_sem = [nc.alloc_semaphore(f"ld{i}") for i in range(6)]
    cast_sem = nc.alloc_semaphore("casts")   # Act increments per cast pair
    stt_sem = nc.alloc_semaphore("stts")     # DVE increments per STT pair
    store_sem = nc.alloc_semaphore("stores")  # store completions

    # ---- loads (sync ring, FIFO): u0 d0 u1 d1 uc dc
    nc.sync.dma_start(u_in[0].ap(), up[0]).then_inc(load_sem[0])
    nc.sync.dma_start(d_in[0].ap(), down[0]).then_inc(load_sem[1])
    nc.sync.dma_start(u_in[1].ap(), up[1]).then_inc(load_sem[2])
    nc.sync.dma_start(d_in[1].ap(), down[1]).then_inc(load_sem[3])
    nc.sync.dma_start(uc_in.ap(), upc.ap()).then_inc(load_sem[4])
    nc.sync.dma_start(dc_in.ap(), downc.ap()).then_inc(load_sem[5])

    def u_ap(t):
        if t < 2:
            return u_in[t].ap()
        return uc_in.ap()[:, (t - 2) * K * W:(t - 1) * K * W]

    def d_ap(t):
        if t < 2:
            return d_in[t].ap()
        return dc_in.ap()[:, (t - 2) * K * W:(t - 1) * K * W]

    # ---- Act: casts for tiles 1-3 (cast sets 1,2,0 -- set0 is tile0's on
    # DVE; tile3 reuses set... sets: tile0->set0(DVE), t1->set1, t2->set2,
    # t3->set1 (reuse; needs STT pair1 done = stt_sem>=2)
    set_of = {0: 0, 1: 1, 2: 2, 3: 1}
    for t in (1, 2, 3):
        s = set_of[t]
        cu = nc.scalar.copy(cast_u[s].ap(), u_ap(t))
        cu.wait_op(load_sem[2] if t == 1 else load_sem[4], 1, "sem-ge")
        if t == 3:
            cu.wait_op(stt_sem, 2, "sem-ge")
        cd = nc.scalar.copy(cast_d[s].ap(), d_ap(t))
        cd.wait_op(load_sem[3] if t == 1 else load_sem[5], 1, "sem-ge")
        cd.then_inc(cast_sem)

    # ---- DVE: tile0 casts + all STT pairs
    cu0 = nc.vector.tensor_copy(cast_u[0].ap(), u_ap(0))
    cu0.wait_op(load_sem[0], 1, "sem-ge")
    cd0 = nc.vector.tensor_copy(cast_d[0].ap(), d_ap(0))
    cd0.wait_op(load_sem[1], 1, "sem-ge")
    for t in range(N_TILES):
        s = set_of[t]
        ob = o_buf[t % 3]
        ov = ob.ap().rearrange("p (k r w) -> p k r w", k=K, r=2, w=W)
        uv = cast_u[s].ap().rearrange("p (k w) -> p k w", k=K)
        dv = cast_d[s].ap().rearrange("p (k w) -> p k w", k=K)
        e = _stt_u32(nc, ov[:, :, 0, :], uv[:], 16, dv[:])
        if t >= 1:
            e.wait_op(cast_sem, t, "sem-ge")
        if t == 3:  # o set0 reused; store0 must have completed
            e.wait_op(store_sem, 1, "sem-ge")
        o = _stt_u32(nc, ov[:, :, 1, :], dv[:], 16, uv[:])
        o.then_inc(stt_sem)

    # ---- stores (sync ring, after all load dispatches)
    for t in range(N_TILES):
        st = nc.sync.dma_start(out[t], o_buf[t % 3].ap())
        st.wait_op(stt_sem, t + 1, "sem-ge")
        st.then_inc(store_sem)

    nc.compile()
    return nc


def _get_nc() -> bass.Bass:
    if not _nc_cache:
        _nc_cache.append(_build_nc())
    return _nc_cache[0]


def _to_bf16_bits(x: np.ndarray) -> np.ndarray:
    u = np.ascontiguousarray(x, dtype=np.float32).view(np.uint32)
    return ((u + np.uint32(0x7FFF) + ((u >> np.uint32(16)) & np.uint32(1)))
            >> np.uint32(16)).astype(np.uint16)


def _from_bf16_bits(y: np.ndarray) -> np.ndarray:
    return (y.astype(np.uint32) << np.uint32(16)).view(np.float32)


def kernel(up_diagonal: np.ndarray, down_diagonal: np.ndarray) -> np.ndarray:
    assert up_diagonal.shape == (B, C, H, W), up_diagonal.shape
    up_bits = _to_bf16_bits(np.asarray(up_diagonal))
    down_bits = _to_bf16_bits(np.asarray(down_diagonal))

    nc = _get_nc()

    def _split(bits, sl):
        rows = bits[sl].reshape(N_TILES, P, K * W)
        fine = rows[:2]
        coarse = np.ascontiguousarray(
            rows[2:].transpose(1, 0, 2)).reshape(P, 2 * K * W)
        return fine, coarse

    in_maps = []
    for core in range(N_CORES):
        sl = slice(core * B_LOC, (core + 1) * B_LOC)
        up_f, up_c = _split(up_bits, sl)
        down_f, down_c = _split(down_bits, sl)
        in_maps.append(
            {"up": up_f, "down": down_f, "upc": up_c, "downc": down_c}
        )

    res = run_bass_kernel_spmd(
        nc, in_maps, core_ids=list(range(N_CORES)), trace=TRACE
    )
    global LAST_RESULT
    LAST_RESULT = res
    results = res.results
    out = np.empty((B, C, 2 * H, 2 * W), dtype=np.float32)
    for core in range(N_CORES):
        sl = slice(core * B_LOC, (core + 1) * B_LOC)
        pairs = results[core]["out"].view(np.uint16)
        out[sl] = _from_bf16_bits(pairs).reshape(B_LOC, C, 2 * H, 2 * W)
    return out


# revision 36
# speedup vs baseline: 1.0536x; 1.0536x over previous
"""Raw-bass (no TileContext) Trainium2 kernel for DiagonalUpsample.

  out[2i,   2j  ] = d[i,j];  out[2i,   2j+1] = u[i,j]
  out[2i+1, 2j  ] = u[i,j];  out[2i+1, 2j+1] = d[i,j]

bf16 end-to-end (gate is 2e-2; bf16 RNE is exact to 2^-8), fused u32
shift-or interleave on DVE, Act/DVE cast split, hand-wired semaphores.
The pipeline is chain-paced (load -> cast -> merge -> store), so loads are
fine-grained per tile and merges/stores run in k-halves: each half-store
dispatches ~1.3us after its half-merge instead of waiting a full pair.
"""

import numpy as np

import concourse.bass as bass
from concourse import bacc, mybir
from concourse.bass_utils import run_bass_kernel_spmd

B, C, H, W = 16, 3, 512, 512
N_CORES = 8
B_LOC = B // N_CORES
ROWS = B_LOC * C * H
P = 128
K = 6
KH = K // 2
N_TILES = ROWS // (P * K)
U16 = mybir.dt.uint16
U32 = mybir.dt.uint32

_nc_cache = []
TRACE = False
LAST_RESULT = None


def _stt_u32(nc, out, in0, shift, in1):
    eng = nc.vector
    return eng.add_instruction(
        mybir.InstTensorScalarPtr(
            name=nc.get_next_instruction_name(),
            is_scalar_tensor_tensor=True,
            op0=mybir.AluOpType.logical_shift_left,
            op1=mybir.AluOpType.bitwise_or,
            ins=[
                eng.lower_ap(in0),
                mybir.ImmediateValue(dtype=mybir.dt.uint32, value=shift),
                eng.lower_ap(in1),
            ],
            outs=[eng.lower_ap(out)],
        )
    )


def _build_nc() -> bass.Bass:
    nc = bacc.Bacc("TRN2", debug=False)
    up = nc.dram_tensor("up", [N_TILES, P, K * W], U16, kind="ExternalInput")
    down = nc.dram_tensor("down", [N_TILES, P, K * W], U16,
                          kind="ExternalInput")
    out = nc.dram_tensor("out", [N_TILES, P, K * 2 * W], U32,
                         kind="ExternalOutput")

    # static SBUF (per partition): 48KB inputs + 3x24KB cast sets +
    # 3x24KB out sets = 192KB
    u_in = [nc.alloc_sbuf_tensor(f"u{t}", [P, K * W], U16)
            for t in range(N_TILES)]
    d_in = [nc.alloc_sbuf_tensor(f"d{t}", [P, K * W], U16)
            for t in range(N_TILES)]
    cast_u = [nc.alloc_sbuf_tensor(f"cu{s}", [P, K * W], U32) for s in range(3)]
    cast_d = [nc.alloc_sbuf_tensor(f"cd{s}", [P, K * W], U32) for s in range(3)]
    o_buf = [nc.alloc_sbuf_tensor(f"o{s}", [P, K * 2 * W], U32)
             for s in range(3)]

    load_sem = [nc.alloc_semaphore(f"ld{i}") for i in range(2 * N_TILES)]
    cast_sem = nc.alloc_semaphore("casts")    # Act: +1 per finished cast pair
    stt_sem = nc.alloc_semaphore("stts")      # DVE: +1 per finished half-pair
    store_sem = nc.alloc_semaphore("stores")  # +16 per half-store completion

    # ---- loads (sync ring, FIFO): u0 d0 u1 d1 u2 d2 u3 d3
    for t in range(N_TILES):
        nc.sync.dma_start(u_in[t].ap(), up[t]).then_inc(load_sem[2 * t], 16)
        nc.sync.dma_start(d_in[t].ap(), down[t]).then_inc(
            load_sem[2 * t + 1], 16)

    # ---- Act: casts for tiles 1-3.  cast sets: t0->0 (DVE), t1->1, t2->2,
    # t3->1 (reuse: wait until tile1's merges -- 4 half-pairs -- are done).
    set_of = {0: 0, 1: 1, 2: 2, 3: 1}
    for t in (1, 2, 3):
        s = set_of[t]
        nc.scalar.wait_ge(load_sem[2 * t], 16)
        if t == 3:
            nc.scalar.wait_ge(stt_sem, 4)
        nc.scalar.copy(cast_u[s].ap(), u_in[t].ap())
        nc.scalar.wait_ge(load_sem[2 * t + 1], 16)
        cd = nc.scalar.copy(cast_d[s].ap(), d_in[t].ap())
        cd.then_inc(cast_sem)

    # ---- DVE: tile0 casts, then all merges in k-halves
    nc.vector.wait_ge(load_sem[0], 16)
    nc.vector.tensor_copy(cast_u[0].ap(), u_in[0].ap())
    nc.vector.wait_ge(load_sem[1], 16)
    nc.vector.tensor_copy(cast_d[0].ap(), d_in[0].ap())
    for t in range(N_TILES):
        s = set_of[t]
        ov = o_buf[t % 3].ap().rearrange("p (k r w) -> p k r w",
                                         k=K, r=2, w=W)
        uv = cast_u[s].ap().rearrange("p (k w) -> p k w", k=K)
        dv = cast_d[s].ap().rearrange("p (k w) -> p k w", k=K)
        if t >= 1:
            nc.vector.wait_ge(cast_sem, t)
        if t == 3:  # o set0 reused: both of store0's halves must be done
            nc.vector.wait_ge(store_sem, 32)
        for h in range(2):
            ks = slice(h * KH, (h + 1) * KH)
            _stt_u32(nc, ov[:, ks, 0, :], uv[:, ks, :], 16, dv[:, ks, :])
            o = _stt_u32(nc, ov[:, ks, 1, :], dv[:, ks, :], 16, uv[:, ks, :])
            o.then_inc(stt_sem)

    # ---- stores (sync ring): one per k-half, chasing the half-merges
    for t in range(N_TILES):
        for h in range(2):
            half = slice(h * K * W, (h + 1) * K * W)
            nc.sync.wait_ge(stt_sem, 2 * t + h + 1)
            st = nc.sync.dma_start(out[t][:, half],
                                   o_buf[t % 3].ap()[:, half])
            st.then_inc(store_sem, 16)

    nc.compile()
    return nc


def _get_nc() -> bass.Bass:
    if not _nc_cache:
        _nc_cache.append(_build_nc())
    return _nc_cache[0]


def _to_bf16_bits(x: np.ndarray) -> np.ndarray:
    u = np.ascontiguousarray(x, dtype=np.float32).view(np.uint32)
    return ((u + np.uint32(0x7FFF) + ((u >> np.uint32(16)) & np.uint32(1)))
            >> np.uint32(16)).astype(np.uint16)


def _from_bf16_bits(y: np.ndarray) -> np.ndarray:
    return (y.astype(np.uint32) << np.uint32(16)).view(np.float32)


def kernel(up_diagonal: np.ndarray, down_diagonal: np.ndarray) -> np.ndarray:
    assert up_diagonal.shape == (B, C, H, W), up_diagonal.shape
    up_bits = _to_bf16_bits(np.asarray(up_diagonal))
    down_bits = _to_bf16_bits(np.asarray(down_diagonal))

    nc = _get_nc()
    in_maps = []
    for core in range(N_CORES):
        sl = slice(core * B_LOC, (core + 1) * B_LOC)
        in_maps.append(
            {
                "up": up_bits[sl].reshape(N_TILES, P, K * W),
                "down": down_bits[sl].reshape(N_TILES, P, K * W),
            }
        )

    res = run_bass_kernel_spmd(
        nc, in_maps, core_ids=list(range(N_CORES)), trace=TRACE
    )
    global LAST_RESULT
    LAST_RESULT = res
    results = res.results
    out = np.empty((B, C, 2 * H, 2 * W), dtype=np.float32)
    for core in range(N_CORES):
        sl = slice(core * B_LOC, (core + 1) * B_LOC)
        pairs = results[core]["out"].view(np.uint16)
        out[sl] = _from_bf16_bits(pairs).reshape(B_LOC, C, 2 * H, 2 * W)
    return out
